# revision 15
# baseline (speedup 1.0000x reference)
"""3-layer GAT on 8 TRN2 NeuronCores.

Sharding: nodes partitioned by dst across 8 cores. Per-core dst nodes are
sorted by degree and packed into groups of 128 (one dst node per SBUF
partition); each node's incoming edges occupy slots along the free axis,
padded to the group max degree. Source rows [h | asrc] are fetched with
dma_gather from a replicated per-layer feature table (AllGather'ed each
layer); adst is one 128-row gather per group (dst is constant per
partition). Softmax is unnormalized (exp, then divide by the reduced e
column). Aggregation = one strided tensor_reduce over the edge-slot axis
— no one-hot matmuls, no PSUM in the aggregation path.

The wall clock is dominated by (a) host->device transfer over the axon
tunnel and (b) a fixed ~50-80us per-instruction dispatch cost, so inputs
are minimized (fp16 x/out, compact int16 gather indices replicated to 128
partitions on device, int32 scatter rows) and the instruction count is
kept low (degree-padded reduce instead of selection matmuls, direct
PSUM->DRAM stores in the dense layers).
"""
import numpy as np

from concourse import bass, bacc, mybir, tile
from concourse.bass_utils import run_bass_kernel_spmd

f32 = mybir.dt.float32
f16 = mybir.dt.float16
i16 = mybir.dt.int16
i32 = mybir.dt.int32
Alu = mybir.AluOpType
Act = mybir.ActivationFunctionType
IOA = bass.IndirectOffsetOnAxis
AX = mybir.AxisListType

FULL_CFG = dict(
    N=50000, IN=128, HID=64, OUT=64, NH=4, E=800000, R=8,
    VSPLIT=32768,                 # int16 index reach for the fat gather
    DCAP=26,                      # max edge slots per gather chunk
    GE=4,                         # groups per merged epilogue
)


def make_cfg(**over):
    cfg = dict(FULL_CFG)
    cfg.update(over)
    N, R = cfg["N"], cfg["R"]
    assert N % R == 0
    cfg["SHARD"] = N // R
    # local rows: shard + >=2 pad rows, multiple of 128
    cfg["LPAD"] = ((cfg["SHARD"] + 2 + 127) // 128) * 128
    cfg["TROWS"] = R * cfg["LPAD"]
    cfg["PADROW"] = cfg["SHARD"]  # local pad row (asrc=-1e30 in every shard)
    cfg["TRASH"] = cfg["LPAD"] - 1
    if cfg["TROWS"] <= cfg["VSPLIT"]:
        cfg["VSPLIT"] = cfg["TROWS"]
    else:
        assert cfg["TROWS"] - cfg["VSPLIT"] <= 32768
        # need a pad row in the high range: core r covers
        # [r*LPAD, r*LPAD+SHARD) real + pads; find r with pad row >= VSPLIT
        r = 0
        while r * cfg["LPAD"] + cfg["SHARD"] < cfg["VSPLIT"]:
            r += 1
        cfg["PADROW_H"] = r * cfg["LPAD"] + cfg["SHARD"]
        assert cfg["PADROW_H"] >= cfg["VSPLIT"]
    cfg["NG"] = (cfg["SHARD"] + 127) // 128   # node groups per core
    # fat table row widths (fp32, multiple of 64 elems = 256B)
    cfg["FATW12"] = 320   # h(256) | asrc(4) | pad
    cfg["FATW3"] = 128    # h(64) | asrc(1) | pad
    cfg["NRW"] = 64       # narrow adst table row width
    return cfg


def _renum(n, cfg):
    return (n // cfg["SHARD"]) * cfg["LPAD"] + (n % cfg["SHARD"])


def _wrap_idx(idx_flat):
    """dma_gather compact int16 index layout: ordinal i at [i%16, i//16]."""
    n = len(idx_flat)
    assert n % 16 == 0
    return np.asarray(idx_flat, np.int16).reshape(n // 16, 16).T


def _group_core(src_g, dst_l, cfg):
    """Group one core's edges: nodes sorted by degree desc, 128 per group.

    Returns (groups, jls, jhs): groups = list of NG dicts with
    nodes [128] (local id, -1 pad), low/high edge lists per node;
    jls/jhs = per-group max low/high counts.
    """
    SHARD, VS, NG = cfg["SHARD"], cfg["VSPLIT"], cfg["NG"]
    order = np.argsort(dst_l, kind="stable")
    src_g = src_g[order]
    dst_l = dst_l[order]
    deg = np.bincount(dst_l, minlength=SHARD)
    starts = np.concatenate([[0], np.cumsum(deg)])
    byd = np.argsort(-deg, kind="stable")
    groups, jls, jhs = [], [], []
    for g in range(NG):
        nodes = byd[g * 128:(g + 1) * 128]
        low, high = [], []
        for n in nodes:
            s = src_g[starts[n]:starts[n + 1]]
            m = s < VS
            low.append(s[m])
            high.append(s[~m])
        nn = np.full(128, -1, np.int64)
        nn[:len(nodes)] = nodes
        pad = 128 - len(nodes)
        low += [np.zeros(0, np.int64)] * pad
        high += [np.zeros(0, np.int64)] * pad
        groups.append(dict(nodes=nn, low=low, high=high))
        jls.append(max(len(v) for v in low))
        jhs.append(max(len(v) for v in high))
    return groups, jls, jhs


def _build_plan(jls, jhs, cfg):
    """Chunk the (cross-core max) per-group slot counts and assign idx
    columns. Returns (plan, nar_base, nix): plan[g] = tuple of
    (jl, jh, off_low, off_high); narrow idx blocks at nar_base + g*8."""
    DCAP = cfg["DCAP"]
    col = 0
    plan = []
    for g in range(cfg["NG"]):
        JL, JH = jls[g], jhs[g]
        D = JL + JH
        chunks = []
        for st in range(0, D, DCAP):
            en = min(D, st + DCAP)
            jl = max(0, min(JL, en) - st)
            jh = (en - st) - jl
            off_l = col
            col += jl * 8
            off_h = col
            col += jh * 8
            chunks.append((jl, jh, off_l, off_h))
        plan.append(tuple(chunks))
    nar_base = col
    col += cfg["NG"] * 8
    return tuple(plan), nar_base, col


def _fill_idx(groups, plan, nar_base, nix, cfg):
    """Build per-core device arrays from groups + shared plan.

    Returns (idxc [16, nix] i16, lid32 [128, NG] i32)."""
    VS = cfg["VSPLIT"]
    PAD_L = cfg["PADROW"]
    PAD_H = cfg.get("PADROW_H", PAD_L)
    TRASH = cfg["TRASH"]
    idxc = np.zeros((16, nix), np.int16)
    lid32 = np.full((128, cfg["NG"]), TRASH, np.int32)
    for g, chunks in enumerate(plan):
        nar_off = nar_base + g * 8
        grp = groups[g]
        jl_done = 0
        jh_done = 0
        for (jl, jh, off_l, off_h) in chunks:
            if jl:
                m = np.full((jl, 128), PAD_L, np.int64)
                for s in range(128):
                    v = grp["low"][s][jl_done:jl_done + jl]
                    m[:len(v), s] = v
                idxc[:, off_l:off_l + jl * 8] = _wrap_idx(m.reshape(-1))
                jl_done += jl
            if jh:
                m = np.full((jh, 128), PAD_H - VS, np.int64)
                for s in range(128):
                    v = grp["high"][s][jh_done:jh_done + jh]
                    m[:len(v), s] = v - VS
                idxc[:, off_h:off_h + jh * 8] = _wrap_idx(m.reshape(-1))
                jh_done += jh
        nar = np.where(grp["nodes"] >= 0, grp["nodes"], TRASH)
        idxc[:, nar_off:nar_off + 8] = _wrap_idx(nar)
        lid32[:, g] = nar
    return idxc, lid32


def prep_host(x, edge_index, cfg):
    """All host-side sharding prep. Returns (per_core_inputs, plan,
    nar_base, nix)."""
    N, R, SHARD, LPAD = cfg["N"], cfg["R"], cfg["SHARD"], cfg["LPAD"]
    IN = cfg["IN"]
    src = np.concatenate([np.asarray(edge_index[0]), np.arange(N)]).astype(np.int64)
    dst = np.concatenate([np.asarray(edge_index[1]), np.arange(N)]).astype(np.int64)
    src_g = _renum(src, cfg)

    per_core_groups = []
    jls = np.zeros(cfg["NG"], np.int64)
    jhs = np.zeros(cfg["NG"], np.int64)
    for r in range(R):
        m = (dst // SHARD) == r
        groups, jl, jh = _group_core(src_g[m], (dst[m] - r * SHARD).astype(np.int64), cfg)
        per_core_groups.append(groups)
        jls = np.maximum(jls, jl)
        jhs = np.maximum(jhs, jh)
    plan, nar_base, nix = _build_plan(jls.tolist(), jhs.tolist(), cfg)

    per_core = []
    for r in range(R):
        idxc, lid32 = _fill_idx(per_core_groups[r], plan, nar_base, nix, cfg)
        xm = np.zeros((IN, LPAD), np.float16)
        xm[:, :SHARD] = np.asarray(x[r * SHARD:(r + 1) * SHARD]).T
        per_core.append(dict(idxc=idxc, lid32=lid32, xmine=xm))
    return per_core, plan, nar_base, nix


def _aug_w(W, a_s, a_d, nh, hid):
    """[inF, outF+2*nh] = [W.T | As | Ad]."""
    inf = W.shape[1]
    Wr = W.reshape(nh, hid, inf)
    As = np.einsum("hci,hc->ih", Wr, a_s)
    Ad = np.einsum("hci,hc->ih", Wr, a_d)
    return np.concatenate([W.T, As, Ad], axis=1).astype(np.float32)


def build_nc(cfg, plan, nar_base, nix):
    N, R = cfg["N"], cfg["R"]
    LPAD, TROWS, SHARD = cfg["LPAD"], cfg["TROWS"], cfg["SHARD"]
    VS, DCAP, NG = cfg["VSPLIT"], cfg["DCAP"], cfg["NG"]
    NH, HID, OUT, IN = cfg["NH"], cfg["HID"], cfg["OUT"], cfg["IN"]
    F = NH * HID              # 256
    FATW, FATW3, NRW = cfg["FATW12"], cfg["FATW3"], cfg["NRW"]
    NLT = LPAD // 128
    npad = LPAD - SHARD

    nc = bacc.Bacc("TRN2", target_bir_lowering=False, debug=False, num_devices=R)

    P = {}
    P["xmine"] = nc.declare_dram_parameter("xmine", [IN, LPAD], f16, isOutput=False)
    P["w1t"] = nc.declare_dram_parameter("w1t", [IN, F + 2 * NH], f16, isOutput=False)
    P["w2t"] = nc.declare_dram_parameter("w2t", [F, F + 2 * NH], f16, isOutput=False)
    P["w3t"] = nc.declare_dram_parameter("w3t", [F, OUT + 2], f16, isOutput=False)
    P["b1"] = nc.declare_dram_parameter("b1", [1, F], f32, isOutput=False)
    P["b2"] = nc.declare_dram_parameter("b2", [1, F], f32, isOutput=False)
    P["b3"] = nc.declare_dram_parameter("b3", [1, OUT], f32, isOutput=False)
    P["idxc"] = nc.declare_dram_parameter("idxc", [16, nix], i16, isOutput=False)
    P["lid32"] = nc.declare_dram_parameter("lid32", [128, NG], i32, isOutput=False)
    out_p = nc.declare_dram_parameter("out", [LPAD, OUT], f16, isOutput=True)

    tbl1 = nc.dram_tensor("tbl1", [TROWS, FATW], f32, addr_space="Shared")
    tbl2 = nc.dram_tensor("tbl2", [TROWS, FATW], f32, addr_space="Shared")
    tbl3 = nc.dram_tensor("tbl3", [TROWS, FATW3], f32, addr_space="Shared")
    own_h1 = nc.dram_tensor("own_h1", [LPAD, FATW], f32)
    own_h2 = nc.dram_tensor("own_h2", [LPAD, FATW], f32)
    own_h3 = nc.dram_tensor("own_h3", [LPAD, FATW3], f32)
    own_x1 = nc.dram_tensor("own_x1", [LPAD, F], f32)
    own_x2 = nc.dram_tensor("own_x2", [LPAD, F], f32)

    with tile.TileContext(nc) as tc:
        with tc.tile_pool(name="const", bufs=1) as cpool, \
             tc.tile_pool(name="work", bufs=3) as wpool, \
             tc.tile_pool(name="gath", bufs=2) as gpool, \
             tc.tile_pool(name="epi", bufs=2) as epool, \
             tc.tile_pool(name="psA", bufs=2, space="PSUM") as psA, \
             tc.tile_pool(name="psB", bufs=2, space="PSUM") as psB:

            def wload(tag, src_ap, w):
                t16 = wpool.tile([128, w], f16, tag="w16")
                nc.sync.dma_start(out=t16[:], in_=src_ap)
                t = cpool.tile([128, w], f32, tag=tag)
                nc.vector.tensor_copy(t[:], t16[:])
                return t

            w1t = wload("w1t", P["w1t"][:], F + 2 * NH)
            w2t_lo = wload("w2lo", P["w2t"][0:128, :], F + 2 * NH)
            w2t_hi = wload("w2hi", P["w2t"][128:256, :], F + 2 * NH)
            w3t_lo = wload("w3lo", P["w3t"][0:128, :], OUT + 2)
            w3t_hi = wload("w3hi", P["w3t"][128:256, :], OUT + 2)

            # biases: ship [1, F], broadcast to 128 partitions by doubling
            def bias_bcast(name, w):
                t = cpool.tile([128, w], f32, tag=name)
                nc.sync.dma_start(out=t[0:1, :], in_=P[name][:])
                p = 1
                while p < 128:
                    nc.sync.dma_start(out=t[p:2 * p, :], in_=t[0:p, :])
                    p *= 2
                return t

            b1 = bias_bcast("b1", F)
            b2 = bias_bcast("b2", F)
            b3 = bias_bcast("b3", OUT)

            # 128x128 identity for PE transposes, via iota
            coli = wpool.tile([128, 128], i32, tag="coli")
            nc.gpsimd.iota(coli[:], [[1, 128]], channel_multiplier=0)
            rowi = wpool.tile([128, 128], i32, tag="rowi")
            nc.gpsimd.iota(rowi[:], [[0, 128]], channel_multiplier=1)
            colf = wpool.tile([128, 128], f32, tag="colf")
            nc.vector.tensor_copy(colf[:], coli[:])
            rowf = wpool.tile([128, 128], f32, tag="rowf")
            nc.vector.tensor_copy(rowf[:], rowi[:])
            ident = cpool.tile([128, 128], f32, tag="ident")
            nc.vector.tensor_tensor(ident[:], colf[:], rowf[:], Alu.is_equal)

            zero = cpool.tile([128, F], f32, tag="zero")
            nc.vector.memset(zero[:], 0.0)
            padc12 = cpool.tile([128, F + 2 * NH], f32, tag="padc12")
            nc.vector.memset(padc12[:], 0.0)
            nc.vector.memset(padc12[:, F:F + NH], -1e30)
            padc3 = cpool.tile([128, OUT + 2], f32, tag="padc3")
            nc.vector.memset(padc3[:], 0.0)
            nc.vector.memset(padc3[:, OUT:OUT + 1], -1e30)

            # resident gather indices: replicate compact [16, nix] to 128 rows
            ixall = cpool.tile([128, nix], i16, tag="ixall")
            for k in range(8):
                nc.sync.dma_start(out=ixall[16 * k:16 * (k + 1), :], in_=P["idxc"][:])
            lid_all = cpool.tile([128, NG], i32, tag="lid")
            nc.sync.dma_start(out=lid_all[:], in_=P["lid32"][:])

            # ---------------- L1 dense (own shard only): [h1|asrc1|adst1] ----
            for c0 in range(0, LPAD, 256):
                cw = min(256, LPAD - c0)
                xc16 = wpool.tile([IN, 256], f16, tag="xc16")
                nc.sync.dma_start(out=xc16[:, 0:cw], in_=P["xmine"][:, c0:c0 + cw])
                xc = wpool.tile([IN, 256], f32, tag="xc")
                nc.vector.tensor_copy(xc[:, 0:cw], xc16[:, 0:cw])
                for s0 in range(0, cw, 128):
                    ps = psA.tile([128, F + 2 * NH], f32, tag="dens")
                    nc.tensor.matmul(ps[:], lhsT=xc[:, s0:s0 + 128], rhs=w1t[:],
                                     start=True, stop=True)
                    hrow = wpool.tile([128, F + 2 * NH], f32, tag="hrow")
                    nc.scalar.activation(hrow[:], ps[:], Act.Copy)
                    nc.sync.dma_start(out=own_h1[c0 + s0:c0 + s0 + 128, 0:F + 2 * NH],
                                      in_=hrow[:])

            def fix_pads(own_h, padc, w):
                nc.sync.dma_start(out=own_h[SHARD:LPAD, 0:w], in_=padc[:npad, :])

            fix_pads(own_h1, padc12, F + 2 * NH)
            nc.gpsimd.collective_compute(
                "AllGather", Alu.bypass, replica_groups=[list(range(R))],
                ins=[own_h1[:].opt()], outs=[tbl1[:].opt()])

            # ---------------- generic agg layer ------------------------------
            GE = cfg["GE"]

            def agg_layer(tbl, own_local, fatw, hw_, nh, c, bias, relu,
                          out_dram, outw, out_dt):
                nhc = nh * c
                payw = nhc + nh
                # one narrow gather per layer: [asrc|adst|pad] rows for all
                # NG groups of own dst nodes; compact adst (cols nh:2nh) into
                # a small resident tile [128, NG, nh]
                gatN = gpool.tile([128, DCAP * fatw], f32, tag="gat")
                nv = gatN[:, 0:NG * NRW].rearrange("p (o q) -> p o q", q=NRW)
                nc.gpsimd.dma_gather(
                    nv, own_local[:, hw_:hw_ + NRW],
                    ixall[:, nar_base:nar_base + NG * 8], NG * 128, NG * 128,
                    NRW, elem_step=fatw, single_packet=False)
                adstC = wpool.tile([128, NG * NH], f32, tag="adstc")
                av_ = adstC[:].rearrange("p (o h) -> p o h", h=NH)
                nc.vector.tensor_copy(av_[:, :, 0:nh], nv[:, :, nh:2 * nh])

                for g0 in range(0, NG, GE):
                    ge = min(GE, NG - g0)
                    accAll = epool.tile([128, GE * payw], f32, tag="acc")
                    for gl in range(ge):
                        g = g0 + gl
                        chunks = plan[g]
                        adst_g = adstC[:, g * NH:g * NH + nh]
                        accsl = accAll[:, gl * payw:(gl + 1) * payw]
                        for ci, (jl, jh, off_l, off_h) in enumerate(chunks):
                            d = jl + jh
                            gat = gpool.tile([128, DCAP * fatw], f32, tag="gat")
                            g3 = gat[:].rearrange("p (j q) -> p j q", q=fatw)
                            if jl:
                                nc.gpsimd.dma_gather(
                                    g3[:, 0:jl, :], tbl[0:VS, :],
                                    ixall[:, off_l:off_l + jl * 8],
                                    jl * 128, jl * 128, fatw, single_packet=False)
                            if jh:
                                nc.gpsimd.dma_gather(
                                    g3[:, jl:d, :], tbl[VS:TROWS, :],
                                    ixall[:, off_h:off_h + jh * 8],
                                    jh * 128, jh * 128, fatw, single_packet=False)
                            # transposed view of the gather: [p, q, j]
                            gq = gat[:].rearrange("p (j q) -> p q j", q=fatw)
                            # logits [p, h, j] = asrc + adst(bcast over j)
                            lgT = wpool.tile([128, nh * DCAP], f32, tag="lgT")
                            lgv = lgT[:].rearrange("p (h j) -> p h j", j=DCAP)
                            nc.vector.tensor_tensor(
                                lgv[:, :, 0:d], gq[:, nhc:nhc + nh, 0:d],
                                adst_g.unsqueeze(2).to_broadcast([128, nh, d]),
                                Alu.add)
                            # leaky_relu(x) = max(x, 0.2x)
                            lgs = wpool.tile([128, nh * DCAP], f32, tag="lgs")
                            lgsv = lgs[:].rearrange("p (h j) -> p h j", j=DCAP)
                            nc.vector.tensor_scalar(lgsv[:, :, 0:d], lgv[:, :, 0:d],
                                                    0.2, None, Alu.mult)
                            lg2 = wpool.tile([128, nh * DCAP], f32, tag="lg2")
                            lg2v = lg2[:].rearrange("p (h j) -> p h j", j=DCAP)
                            nc.vector.tensor_tensor(lg2v[:, :, 0:d], lgv[:, :, 0:d],
                                                    lgsv[:, :, 0:d], Alu.max)
                            # payT [p, payw, j]: h*e rows then e rows
                            payT = gpool.tile([128, payw * DCAP], f32, tag="payT")
                            pv = payT[:].rearrange("p (q j) -> p q j", j=DCAP)
                            ev = pv[:, nhc:nhc + nh, :]
                            nc.scalar.activation(ev[:, :, 0:d], lg2v[:, :, 0:d],
                                                 Act.Exp)
                            pn = payT[:, 0:nhc * DCAP].rearrange(
                                "p (h cc j) -> p h cc j", cc=c, j=DCAP)
                            hq = gq[:, 0:nhc, :].rearrange(
                                "p (h cc) j -> p h cc j", cc=c)
                            nc.vector.tensor_tensor(
                                pn[:, :, :, 0:d], hq[:, :, :, 0:d],
                                ev[:, :, 0:d].unsqueeze(2).to_broadcast(
                                    [128, nh, c, d]),
                                Alu.mult)
                            if ci == 0:
                                nc.vector.tensor_reduce(accsl, pv[:, :, 0:d],
                                                        AX.X, Alu.add)
                            else:
                                tmp = wpool.tile([128, payw], f32, tag="tmp")
                                nc.vector.tensor_reduce(tmp[:], pv[:, :, 0:d],
                                                        AX.X, Alu.add)
                                nc.vector.tensor_tensor(accsl, accsl, tmp[:],
                                                        Alu.add)
                    # merged epilogue over ge groups
                    av = accAll[:].rearrange("p (g q) -> p g q", q=payw)
                    rden = epool.tile([128, GE * nh], f32, tag="rden")
                    rv = rden[:].rearrange("p (g h) -> p g h", h=nh)
                    nc.vector.reciprocal(rv[:, 0:ge, :],
                                         av[:, 0:ge, nhc:nhc + nh])
                    ob = epool.tile([128, GE * outw], f32, tag="ob")
                    obv = ob[:].rearrange("p (g h q) -> p g h q", h=nh, q=c)
                    num = av[:, 0:ge, 0:nhc].rearrange("p g (h q) -> p g h q", q=c)
                    nc.vector.tensor_tensor(
                        obv[:, 0:ge], num,
                        rv[:, 0:ge, :].unsqueeze(3).to_broadcast([128, ge, nh, c]),
                        Alu.mult)
                    o2v = ob[:].rearrange("p (g q) -> p g q", q=outw)
                    nc.vector.tensor_tensor(
                        o2v[:, 0:ge, :], o2v[:, 0:ge, :],
                        bias[:, :outw].unsqueeze(1).to_broadcast([128, ge, outw]),
                        Alu.add)
                    ofin = epool.tile([128, GE * outw], out_dt, tag="ofin")
                    nc.scalar.activation(ofin[:, 0:ge * outw], ob[:, 0:ge * outw],
                                         Act.Relu if relu else Act.Copy)
                    for gl in range(ge):
                        nc.gpsimd.indirect_dma_start(
                            out=out_dram[:],
                            out_offset=IOA(ap=lid_all[:, g0 + gl:g0 + gl + 1],
                                           axis=0),
                            in_=ofin[:, gl * outw:(gl + 1) * outw], in_offset=None)

            # ---------------- own-shard dense (L2/L3) ------------------------
            def dense_own(x_dram, wlo, whi, own_h, nasrc):
                nw = wlo.shape[1]
                for t in range(NLT):
                    xr = wpool.tile([128, F], f32, tag="xr")
                    nc.sync.dma_start(out=xr[:], in_=x_dram[t * 128:(t + 1) * 128, :])
                    pt0 = psB.tile([128, 128], f32, tag="tr")
                    nc.tensor.transpose(out=pt0[:], in_=xr[:, 0:128], identity=ident[:])
                    xT0 = wpool.tile([128, 128], f32, tag="xT0")
                    nc.scalar.activation(xT0[:], pt0[:], Act.Copy)
                    pt1 = psB.tile([128, 128], f32, tag="tr")
                    nc.tensor.transpose(out=pt1[:], in_=xr[:, 128:256], identity=ident[:])
                    xT1 = wpool.tile([128, 128], f32, tag="xT1")
                    nc.scalar.activation(xT1[:], pt1[:], Act.Copy)
                    ps = psA.tile([128, nw], f32, tag="dens")
                    nc.tensor.matmul(ps[:], lhsT=xT0[:], rhs=wlo[:], start=True, stop=False)
                    nc.tensor.matmul(ps[:], lhsT=xT1[:], rhs=whi[:], start=False, stop=True)
                    hrow = wpool.tile([128, nw], f32, tag="hrow")
                    nc.scalar.activation(hrow[:], ps[:], Act.Copy)
                    nc.sync.dma_start(out=own_h[t * 128:(t + 1) * 128, 0:nw],
                                      in_=hrow[:])

            # ================= pipeline =================
            # L1 agg -> own_x1
            nc.sync.dma_start(out=own_x1[SHARD:LPAD, :], in_=zero[:npad, :])
            agg_layer(tbl1, own_h1, FATW, F, NH, HID, b1, True, own_x1, F, f32)

            # L2 dense -> own_h2, fix pads, allgather -> tbl2
            dense_own(own_x1, w2t_lo, w2t_hi, own_h2, NH)
            fix_pads(own_h2, padc12, F + 2 * NH)
            nc.gpsimd.collective_compute(
                "AllGather", Alu.bypass, replica_groups=[list(range(R))],
                ins=[own_h2[:].opt()], outs=[tbl2[:].opt()])

            # L2 agg -> own_x2
            nc.sync.dma_start(out=own_x2[SHARD:LPAD, :], in_=zero[:npad, :])
            agg_layer(tbl2, own_h2, FATW, F, NH, HID, b2, True, own_x2, F, f32)

            # L3 dense -> own_h3, fix pads, allgather -> tbl3
            dense_own(own_x2, w3t_lo, w3t_hi, own_h3, 1)
            fix_pads(own_h3, padc3, OUT + 2)
            nc.gpsimd.collective_compute(
                "AllGather", Alu.bypass, replica_groups=[list(range(R))],
                ins=[own_h3[:].opt()], outs=[tbl3[:].opt()])

            # L3 agg -> out (fp16)
            agg_layer(tbl3, own_h3, FATW3, OUT, 1, OUT, b3, False, out_p, OUT, f16)

    if not nc.is_finalized():
        nc.finalize()
    return nc


def make_inputs(inputs, cfg):
    """Host prep: returns (nc-ready in_maps list, plan, nar_base, nix)."""
    x = np.asarray(inputs["x"], np.float32)
    edge_index = np.asarray(inputs["edge_index"])
    NH, HID, OUT = cfg["NH"], cfg["HID"], cfg["OUT"]
    per_core, plan, nar_base, nix = prep_host(x, edge_index, cfg)

    w1t = _aug_w(np.asarray(inputs["W1"], np.float32),
                 np.asarray(inputs["as1"], np.float32),
                 np.asarray(inputs["ad1"], np.float32), NH, HID)
    w2t = _aug_w(np.asarray(inputs["W2"], np.float32),
                 np.asarray(inputs["as2"], np.float32),
                 np.asarray(inputs["ad2"], np.float32), NH, HID)
    w3t = _aug_w(np.asarray(inputs["W3"], np.float32),
                 np.asarray(inputs["as3"], np.float32),
                 np.asarray(inputs["ad3"], np.float32), 1, OUT)
    w1t = w1t.astype(np.float16)
    w2t = w2t.astype(np.float16)
    w3t = w3t.astype(np.float16)
    F = NH * HID
    b1 = np.asarray(inputs["b1"], np.float32).reshape(1, F)
    b2 = np.asarray(inputs["b2"], np.float32).reshape(1, F)
    b3 = np.asarray(inputs["b3"], np.float32).reshape(1, OUT)

    shared = dict(w1t=w1t, w2t=w2t, w3t=w3t, b1=b1, b2=b2, b3=b3)
    in_maps = []
    for r in range(cfg["R"]):
        m = dict(shared)
        m.update(per_core[r])
        in_maps.append(m)
    return in_maps, plan, nar_base, nix


_KERNEL_CACHE = {}


def run(inputs, cfg=None, trace=False):
    cfg = cfg or make_cfg()
    in_maps, plan, nar_base, nix = make_inputs(inputs, cfg)
    key = (cfg["N"], cfg["E"], plan)
    if key not in _KERNEL_CACHE:
        _KERNEL_CACHE[key] = build_nc(cfg, plan, nar_base, nix)
    nc = _KERNEL_CACHE[key]
    res = run_bass_kernel_spmd(nc, in_maps, list(range(cfg["R"])), trace=trace)
    outs = [res.results[r]["out"][:cfg["SHARD"]] for r in range(cfg["R"])]
    return np.concatenate(outs, axis=0).astype(np.float32), res


def kernel(**inputs):
    out, _ = run(inputs)
    return out
